# revision 18
# baseline (speedup 1.0000x reference)
import sys

sys.path.insert(0, "/opt/trn_rl_repo")

from contextlib import ExitStack

import numpy as np

import concourse.bacc as bacc
import concourse.bass as bass
import concourse.mybir as mybir
import concourse.tile as tile
from concourse.bass_utils import run_bass_kernel_spmd
from concourse.masks import make_identity

NCORES = 8
BZ = 32          # full batch
B = BZ // NCORES  # batches per core
S = 1024
T = 1024
D = 512
E = 2 * D
NEG = -30000.0

f32 = mybir.dt.float32
f32r = mybir.dt.float32r
Exp = mybir.ActivationFunctionType.Exp
Ln = mybir.ActivationFunctionType.Ln
X = mybir.AxisListType.X
MIN = mybir.AluOpType.min


def build():
    nc = bacc.Bacc("TRN2", target_bir_lowering=False, debug=False, num_devices=NCORES)

    srcT = nc.declare_dram_parameter("srcT", [B, D, S], f32r, isOutput=False)
    tgtT = nc.declare_dram_parameter("tgtT", [B, D, T], f32r, isOutput=False)
    srcN = nc.declare_dram_parameter("srcN", [B, S, D], f32r, isOutput=False)
    wT = nc.declare_dram_parameter("wT", [E, D], f32r, isOutput=False)
    mbias = nc.declare_dram_parameter("mbias", [B, S], f32, isOutput=False)
    mbias_pp = nc.declare_dram_parameter("mbias_pp", [B, 128, 8], f32, isOutput=False)
    align = nc.declare_dram_parameter("align", [B, T, S], f32, isOutput=True)
    attnh = nc.declare_dram_parameter("attnh", [B, T, D], f32, isOutput=True)

    with tile.TileContext(nc) as tc, ExitStack() as ctx:
        wp = ctx.enter_context(tc.tile_pool(name="wp", bufs=1))
        inp = ctx.enter_context(tc.tile_pool(name="inp", bufs=2))
        inpt = ctx.enter_context(tc.tile_pool(name="inpt", bufs=3))
        inp1 = ctx.enter_context(tc.tile_pool(name="inp1", bufs=1))
        bc = ctx.enter_context(tc.tile_pool(name="bc", bufs=2))
        sc = ctx.enter_context(tc.tile_pool(name="sc", bufs=2))
        mrp = ctx.enter_context(tc.tile_pool(name="mrp", bufs=2))
        alo = ctx.enter_context(tc.tile_pool(name="alo", bufs=2))
        alt = ctx.enter_context(tc.tile_pool(name="alt", bufs=1))
        ctp = ctx.enter_context(tc.tile_pool(name="ctp", bufs=1))
        outp = ctx.enter_context(tc.tile_pool(name="outp", bufs=2))
        st = ctx.enter_context(tc.tile_pool(name="st", bufs=2))
        dr = ctx.enter_context(tc.tile_pool(name="dr", bufs=2, space="DRAM"))
        psAL = ctx.enter_context(tc.tile_pool(name="psAL", bufs=3, space="PSUM"))
        psB = ctx.enter_context(tc.tile_pool(name="psB", bufs=3, space="PSUM"))
        psC = ctx.enter_context(tc.tile_pool(name="psC", bufs=2, space="PSUM"))

        state = {}

        def emit_loads(b):
            sT = srcT[b, :, :]
            tT = tgtT[b, :, :]
            mb = mbias[b, :]
            mask_bc = mrp.tile([128, S], f32, tag="mask_bc", name=f"mask_bc{b}")
            nc.gpsimd.dma_start(
                out=mask_bc,
                in_=bass.AP(tensor=mb.tensor, offset=mb.offset,
                            ap=[[0, 128], [1, S]]),
            )
            mask_pp = st.tile([128, 8], f32, tag="mask_pp", name=f"mask_pp{b}")
            nc.gpsimd.dma_start(out=mask_pp, in_=mbias_pp[b, :, :])
            # srcT_sb[p, k, s] = src.T[k*128+p, s]   (4 d-chunks)
            srcT_sb = inp.tile([128, 4, S], f32r, tag="srcT", name=f"srcT{b}")
            tgtT_sb = inpt.tile([128, 4, T], f32r, tag="tgtT", name=f"tgtT{b}")
            for k in range(4):
                nc.gpsimd.dma_start(
                    out=srcT_sb[:, k, :],
                    in_=bass.AP(tensor=sT.tensor, offset=sT.offset + k * 128 * S,
                                ap=[[S, 128], [1, S]]),
                )
                nc.gpsimd.dma_start(
                    out=tgtT_sb[:, k, :],
                    in_=bass.AP(tensor=tT.tensor, offset=tT.offset + k * 128 * T,
                                ap=[[T, 128], [1, T]]),
                )
            state[b] = dict(srcT=srcT_sb, tgtT=tgtT_sb, mask_bc=mask_bc, mask_pp=mask_pp)

        def emit_srcn_load(b):
            sN = srcN[b, :, :]
            # srcN_sb[p, k, d] = src[k*128+p, d]   (8 s-chunks)
            srcN_sb = inp1.tile([128, 8, D], f32r, tag="srcN", name=f"srcN{b}")
            nc.gpsimd.dma_start(
                out=srcN_sb,
                in_=bass.AP(tensor=sN.tensor, offset=sN.offset,
                            ap=[[D, 128], [128 * D, 8], [1, D]]),
            )
            state[b]["srcN"] = srcN_sb

        def emit_pass_a_chunk(b, i):
            s = state[b]
            srcT_sb, tgtT_sb, mask_bc = s["srcT"], s["tgtT"], s["mask_bc"]
            negmaxpk, rsumpk = s["negmaxpk"], s["rsumpk"]
            ps = [psAL.tile([128, 512], f32, tag="psAL", name=f"psA{h}")
                  for h in range(2)]
            for h in range(2):
                for k in range(4):
                    nc.tensor.matmul(
                        ps[h],
                        tgtT_sb[:, k, i * 128:(i + 1) * 128],
                        srcT_sb[:, k, h * 512:(h + 1) * 512],
                        start=(k == 0),
                        stop=(k == 3),
                    )
            score = sc.tile([128, S], f32, tag="e1")
            for h in range(2):
                nc.vector.tensor_add(
                    score[:, h * 512:(h + 1) * 512], ps[h],
                    mask_bc[:, h * 512:(h + 1) * 512],
                )
            nc.vector.reduce_max(out=negmaxpk[:, i:i + 1], in_=score,
                                 axis=X, negate=True)
            nc.scalar.activation(out=score, in_=score, func=Exp,
                                 bias=negmaxpk[:, i:i + 1],
                                 accum_out=rsumpk[:, i:i + 1])
            recip = st.tile([128, 1], f32, tag="recip")
            nc.vector.reciprocal(out=recip, in_=rsumpk[:, i:i + 1])
            alout = alo.tile([128, S], f32, tag="alout")
            if i % 2 == 0:
                nc.vector.tensor_scalar_mul(out=alout, in0=score, scalar1=recip)
            else:
                nc.scalar.mul(out=alout, in_=score, mul=recip)
            nc.sync.dma_start(out=align[b, i * 128:(i + 1) * 128, :], in_=alout)

        def emit_adj_chain(b):
            s = state[b]
            negmaxpk, rsumpk = s["negmaxpk"], s["rsumpk"]
            lnzpk = st.tile([128, 8], f32, tag="lnzpk", name=f"lnzpk{b}")
            nc.scalar.activation(out=lnzpk, in_=rsumpk, func=Ln)
            adjpk = st.tile([128, 8], f32, tag="adjpk", name=f"adjpk{b}")
            nc.vector.tensor_sub(adjpk, negmaxpk, lnzpk)
            trps = psC.tile([8, 128], f32, tag="psC", name=f"psT{b}")
            nc.tensor.transpose(trps, adjpk, state["ident"])
            adjT = st.tile([8, 128], f32, tag="adjT", name=f"adjT{b}")
            nc.vector.tensor_copy(adjT, trps)
            adjd = dr.tile([T], f32, tag="adjd", name=f"adjd{b}")
            nc.sync.dma_start(
                out=bass.AP(tensor=adjd.tensor, offset=adjd.offset,
                            ap=[[128, 8], [1, 128]]),
                in_=adjT,
            )
            adj_bc = bc.tile([128, T], f32, tag="adj_bc", name=f"adj_bc{b}")
            nc.sync.dma_start(
                out=adj_bc,
                in_=bass.AP(tensor=adjd.tensor, offset=adjd.offset,
                            ap=[[0, 128], [1, T]]),
            )
            s["adj_bc"] = adj_bc

        def emit_pass_b_chunk(b, j):
            s = state[b]
            srcT_sb, tgtT_sb = s["srcT"], s["tgtT"]
            adj_bc, mask_pp = s["adj_bc"], s["mask_pp"]
            alignT_sb = s["alignT"]
            ps = [psB.tile([128, 512], f32, tag="psB", name=f"psB{h}")
                  for h in range(2)]
            for h in range(2):
                for k in range(4):
                    nc.tensor.matmul(
                        ps[h],
                        srcT_sb[:, k, j * 128:(j + 1) * 128],
                        tgtT_sb[:, k, h * 512:(h + 1) * 512],
                        start=(k == 0),
                        stop=(k == 3),
                    )
            scoreB = sc.tile([128, T], f32, tag="scoreB")
            for h in range(2):
                nc.vector.tensor_add(
                    scoreB[:, h * 512:(h + 1) * 512], ps[h],
                    adj_bc[:, h * 512:(h + 1) * 512],
                )
            nc.scalar.activation(out=alignT_sb[:, j, :], in_=scoreB, func=Exp,
                                 bias=mask_pp[:, j:j + 1])

        def emit_context(b):
            s = state[b]
            srcN_sb, alignT_sb = s["srcN"], s["alignT"]
            cT_sb = ctp.tile([128, 4, T], f32r, tag="cT", name=f"cT{b}")
            for di in range(4):
                for h in range(2):
                    pc = psC.tile([128, 512], f32, tag="psC", name="psC")
                    for k in range(8):
                        nc.tensor.matmul(
                            pc,
                            srcN_sb[:, k, di * 128:(di + 1) * 128],
                            alignT_sb[:, k, h * 512:(h + 1) * 512],
                            start=(k == 0),
                            stop=(k == 7),
                        )
                    nc.vector.tensor_copy(out=cT_sb[:, di, h * 512:(h + 1) * 512], in_=pc)
            s["cT"] = cT_sb

        def emit_linear(b):
            s = state[b]
            cT_sb, tgtT_sb = s["cT"], s["tgtT"]
            wT_sb = state["wT"]
            for i in range(8):  # t-chunks
                pl = psAL.tile([128, 512], f32, tag="psAL", name="psL")
                for n, ek in enumerate([4, 5, 6, 7, 0, 1, 2, 3]):
                    if ek < 4:
                        lhsT = cT_sb[:, ek, i * 128:(i + 1) * 128]
                    else:
                        lhsT = tgtT_sb[:, ek - 4, i * 128:(i + 1) * 128]
                    nc.tensor.matmul(pl, lhsT, wT_sb[:, ek, :],
                                     start=(n == 0), stop=(n == 7))
                ao = outp.tile([128, D], f32, tag="attn_out")
                nc.scalar.copy(out=ao, in_=pl)
                nc.sync.dma_start(out=attnh[b, i * 128:(i + 1) * 128, :], in_=ao)

        # one-batch software pipeline, with pass A(it) and pass B(it-1)
        # interleaved at chunk granularity
        for it in range(B + 1):
            if it < B:
                emit_loads(it)
            if it == 0:
                ident = wp.tile([128, 128], f32)
                make_identity(nc, ident)
                state["ident"] = ident
            if it == 1:
                wT_sb = wp.tile([128, 8, D], f32r)
                nc.gpsimd.dma_start(
                    out=wT_sb,
                    in_=bass.AP(tensor=wT[:, :].tensor, offset=0,
                                ap=[[D, 128], [128 * D, 8], [1, D]]),
                )
                state["wT"] = wT_sb
            if it < B:
                state[it]["negmaxpk"] = st.tile([128, 8], f32, tag="negmaxpk",
                                                name=f"negmaxpk{it}")
                state[it]["rsumpk"] = st.tile([128, 8], f32, tag="rsumpk",
                                              name=f"rsumpk{it}")
            if it > 0:
                state[it - 1]["alignT"] = alt.tile([128, 8, T], f32r, tag="alignT",
                                                   name=f"alignT{it - 1}")
                emit_srcn_load(it - 1)
            for i in range(8):
                if it < B:
                    emit_pass_a_chunk(it, i)
                if it > 0:
                    emit_pass_b_chunk(it - 1, i)
            if it < B:
                emit_adj_chain(it)
            if it > 0:
                b = it - 1
                emit_context(b)
                emit_linear(b)
                del state[b]

    nc.compile()
    return nc


_NC = None


def _get_nc():
    global _NC
    if _NC is None:
        _NC = build()
    return _NC


def make_in_maps(src, tgt, src_lengths, w_out):
    src = np.ascontiguousarray(src, dtype=np.float32)
    tgt = np.ascontiguousarray(tgt, dtype=np.float32)
    srcT = np.ascontiguousarray(src.transpose(0, 2, 1))
    tgtT = np.ascontiguousarray(tgt.transpose(0, 2, 1))
    wT = np.ascontiguousarray(np.asarray(w_out, dtype=np.float32).T)
    lengths = np.asarray(src_lengths)
    mb = np.where(np.arange(S)[None, :] < lengths[:, None], 0.0, NEG).astype(np.float32)
    # mb_pp[b, p, j] = mb[b, j*128 + p]
    mb_pp = np.ascontiguousarray(mb.reshape(BZ, 8, 128).transpose(0, 2, 1))
    in_maps = []
    for c in range(NCORES):
        sl = slice(c * B, (c + 1) * B)
        in_maps.append({
            "srcT": srcT[sl],
            "tgtT": tgtT[sl],
            "srcN": src[sl],
            "wT": wT,
            "mbias": mb[sl],
            "mbias_pp": mb_pp[sl],
        })
    return in_maps


def kernel(src, tgt, src_lengths, w_out, **run_kwargs):
    nc = _get_nc()
    in_maps = make_in_maps(src, tgt, src_lengths, w_out)
    res = run_bass_kernel_spmd(nc, in_maps, list(range(NCORES)), **run_kwargs)
    attn_h = np.concatenate([res.results[c]["attnh"] for c in range(NCORES)], axis=0)
    align = np.concatenate([res.results[c]["align"] for c in range(NCORES)], axis=0)
    if run_kwargs:
        kernel.last_results = res
    return attn_h, align


# revision 19
# speedup vs baseline: 1.1093x; 1.1093x over previous
import sys

sys.path.insert(0, "/opt/trn_rl_repo")

from contextlib import ExitStack

import numpy as np

import concourse.bacc as bacc
import concourse.bass as bass
import concourse.mybir as mybir
import concourse.tile as tile
from concourse.bass_utils import run_bass_kernel_spmd
from concourse.masks import make_identity

NCORES = 8
BZ = 32          # full batch
B = BZ // NCORES  # batches per core
S = 1024
T = 1024
D = 512
E = 2 * D
NEG = -30000.0

f32 = mybir.dt.float32
f32r = mybir.dt.float32r
Exp = mybir.ActivationFunctionType.Exp
Ln = mybir.ActivationFunctionType.Ln
X = mybir.AxisListType.X
MIN = mybir.AluOpType.min


def build():
    nc = bacc.Bacc("TRN2", target_bir_lowering=False, debug=False, num_devices=NCORES)

    srcT = nc.declare_dram_parameter("srcT", [B, D, S], f32r, isOutput=False)
    tgtT = nc.declare_dram_parameter("tgtT", [B, D, T], f32r, isOutput=False)
    srcN = nc.declare_dram_parameter("srcN", [B, S, D], f32r, isOutput=False)
    wT = nc.declare_dram_parameter("wT", [E, D], f32r, isOutput=False)
    mbias = nc.declare_dram_parameter("mbias", [B, S], f32, isOutput=False)
    mbias_pp = nc.declare_dram_parameter("mbias_pp", [B, 128, 8], f32, isOutput=False)
    align = nc.declare_dram_parameter("align", [B, T, S], f32, isOutput=True)
    attnh = nc.declare_dram_parameter("attnh", [B, T, D], f32, isOutput=True)

    with tile.TileContext(nc) as tc, ExitStack() as ctx:
        wp = ctx.enter_context(tc.tile_pool(name="wp", bufs=1))
        inp = ctx.enter_context(tc.tile_pool(name="inp", bufs=2))
        inpt = ctx.enter_context(tc.tile_pool(name="inpt", bufs=3))
        inp1 = ctx.enter_context(tc.tile_pool(name="inp1", bufs=1))
        bc = ctx.enter_context(tc.tile_pool(name="bc", bufs=2))
        sc = ctx.enter_context(tc.tile_pool(name="sc", bufs=2))
        mrp = ctx.enter_context(tc.tile_pool(name="mrp", bufs=2))
        alo = ctx.enter_context(tc.tile_pool(name="alo", bufs=2))
        alt = ctx.enter_context(tc.tile_pool(name="alt", bufs=1))
        ctp = ctx.enter_context(tc.tile_pool(name="ctp", bufs=1))
        outp = ctx.enter_context(tc.tile_pool(name="outp", bufs=2))
        st = ctx.enter_context(tc.tile_pool(name="st", bufs=2))
        dr = ctx.enter_context(tc.tile_pool(name="dr", bufs=2, space="DRAM"))
        psAL = ctx.enter_context(tc.tile_pool(name="psAL", bufs=3, space="PSUM"))
        psB = ctx.enter_context(tc.tile_pool(name="psB", bufs=3, space="PSUM"))
        psC = ctx.enter_context(tc.tile_pool(name="psC", bufs=2, space="PSUM"))

        state = {}

        def emit_loads(b):
            sT = srcT[b, :, :]
            tT = tgtT[b, :, :]
            mb = mbias[b, :]
            mask_bc = mrp.tile([128, S], f32, tag="mask_bc", name=f"mask_bc{b}")
            nc.sync.dma_start(
                out=mask_bc,
                in_=bass.AP(tensor=mb.tensor, offset=mb.offset,
                            ap=[[0, 128], [1, S]]),
            )
            mask_pp = st.tile([128, 8], f32, tag="mask_pp", name=f"mask_pp{b}")
            nc.sync.dma_start(out=mask_pp, in_=mbias_pp[b, :, :])
            # srcT_sb[p, k, s] = src.T[k*128+p, s]   (4 d-chunks)
            srcT_sb = inp.tile([128, 4, S], f32r, tag="srcT", name=f"srcT{b}")
            tgtT_sb = inpt.tile([128, 4, T], f32r, tag="tgtT", name=f"tgtT{b}")
            for k in range(4):
                nc.sync.dma_start(
                    out=srcT_sb[:, k, :],
                    in_=bass.AP(tensor=sT.tensor, offset=sT.offset + k * 128 * S,
                                ap=[[S, 128], [1, S]]),
                )
                nc.sync.dma_start(
                    out=tgtT_sb[:, k, :],
                    in_=bass.AP(tensor=tT.tensor, offset=tT.offset + k * 128 * T,
                                ap=[[T, 128], [1, T]]),
                )
            state[b] = dict(srcT=srcT_sb, tgtT=tgtT_sb, mask_bc=mask_bc, mask_pp=mask_pp)

        def emit_srcn_load(b):
            sN = srcN[b, :, :]
            # srcN_sb[p, k, d] = src[k*128+p, d]   (8 s-chunks)
            srcN_sb = inp1.tile([128, 8, D], f32r, tag="srcN", name=f"srcN{b}")
            nc.sync.dma_start(
                out=srcN_sb,
                in_=bass.AP(tensor=sN.tensor, offset=sN.offset,
                            ap=[[D, 128], [128 * D, 8], [1, D]]),
            )
            state[b]["srcN"] = srcN_sb

        def emit_pass_a_chunk(b, i):
            s = state[b]
            srcT_sb, tgtT_sb, mask_bc = s["srcT"], s["tgtT"], s["mask_bc"]
            negmaxpk, rsumpk = s["negmaxpk"], s["rsumpk"]
            ps = [psAL.tile([128, 512], f32, tag="psAL", name=f"psA{h}")
                  for h in range(2)]
            for h in range(2):
                for k in range(4):
                    nc.tensor.matmul(
                        ps[h],
                        tgtT_sb[:, k, i * 128:(i + 1) * 128],
                        srcT_sb[:, k, h * 512:(h + 1) * 512],
                        start=(k == 0),
                        stop=(k == 3),
                    )
            score = sc.tile([128, S], f32, tag="e1")
            for h in range(2):
                nc.vector.tensor_add(
                    score[:, h * 512:(h + 1) * 512], ps[h],
                    mask_bc[:, h * 512:(h + 1) * 512],
                )
            nc.vector.reduce_max(out=negmaxpk[:, i:i + 1], in_=score,
                                 axis=X, negate=True)
            nc.scalar.activation(out=score, in_=score, func=Exp,
                                 bias=negmaxpk[:, i:i + 1],
                                 accum_out=rsumpk[:, i:i + 1])
            recip = st.tile([128, 1], f32, tag="recip")
            nc.vector.reciprocal(out=recip, in_=rsumpk[:, i:i + 1])
            alout = alo.tile([128, S], f32, tag="alout")
            if i % 2 == 0:
                nc.vector.tensor_scalar_mul(out=alout, in0=score, scalar1=recip)
            else:
                nc.scalar.mul(out=alout, in_=score, mul=recip)
            nc.sync.dma_start(out=align[b, i * 128:(i + 1) * 128, :], in_=alout)

        def emit_adj_chain(b):
            s = state[b]
            negmaxpk, rsumpk = s["negmaxpk"], s["rsumpk"]
            lnzpk = st.tile([128, 8], f32, tag="lnzpk", name=f"lnzpk{b}")
            nc.scalar.activation(out=lnzpk, in_=rsumpk, func=Ln)
            adjpk = st.tile([128, 8], f32, tag="adjpk", name=f"adjpk{b}")
            nc.vector.tensor_sub(adjpk, negmaxpk, lnzpk)
            trps = psC.tile([8, 128], f32, tag="psC", name=f"psT{b}")
            nc.tensor.transpose(trps, adjpk, state["ident"])
            adjT = st.tile([8, 128], f32, tag="adjT", name=f"adjT{b}")
            nc.vector.tensor_copy(adjT, trps)
            adjd = dr.tile([T], f32, tag="adjd", name=f"adjd{b}")
            nc.sync.dma_start(
                out=bass.AP(tensor=adjd.tensor, offset=adjd.offset,
                            ap=[[128, 8], [1, 128]]),
                in_=adjT,
            )
            adj_bc = bc.tile([128, T], f32, tag="adj_bc", name=f"adj_bc{b}")
            nc.sync.dma_start(
                out=adj_bc,
                in_=bass.AP(tensor=adjd.tensor, offset=adjd.offset,
                            ap=[[0, 128], [1, T]]),
            )
            s["adj_bc"] = adj_bc

        def emit_pass_b_chunk(b, j):
            s = state[b]
            srcT_sb, tgtT_sb = s["srcT"], s["tgtT"]
            adj_bc, mask_pp = s["adj_bc"], s["mask_pp"]
            alignT_sb = s["alignT"]
            ps = [psB.tile([128, 512], f32, tag="psB", name=f"psB{h}")
                  for h in range(2)]
            for h in range(2):
                for k in range(4):
                    nc.tensor.matmul(
                        ps[h],
                        srcT_sb[:, k, j * 128:(j + 1) * 128],
                        tgtT_sb[:, k, h * 512:(h + 1) * 512],
                        start=(k == 0),
                        stop=(k == 3),
                    )
            scoreB = sc.tile([128, T], f32, tag="scoreB")
            for h in range(2):
                nc.vector.tensor_add(
                    scoreB[:, h * 512:(h + 1) * 512], ps[h],
                    adj_bc[:, h * 512:(h + 1) * 512],
                )
            nc.scalar.activation(out=alignT_sb[:, j, :], in_=scoreB, func=Exp,
                                 bias=mask_pp[:, j:j + 1])

        def emit_context(b):
            s = state[b]
            srcN_sb, alignT_sb = s["srcN"], s["alignT"]
            cT_sb = ctp.tile([128, 4, T], f32r, tag="cT", name=f"cT{b}")
            for di in range(4):
                for h in range(2):
                    pc = psC.tile([128, 512], f32, tag="psC", name="psC")
                    for k in range(8):
                        nc.tensor.matmul(
                            pc,
                            srcN_sb[:, k, di * 128:(di + 1) * 128],
                            alignT_sb[:, k, h * 512:(h + 1) * 512],
                            start=(k == 0),
                            stop=(k == 7),
                        )
                    nc.vector.tensor_copy(out=cT_sb[:, di, h * 512:(h + 1) * 512], in_=pc)
            s["cT"] = cT_sb

        def emit_linear(b):
            s = state[b]
            cT_sb, tgtT_sb = s["cT"], s["tgtT"]
            wT_sb = state["wT"]
            for i in range(8):  # t-chunks
                pl = psAL.tile([128, 512], f32, tag="psAL", name="psL")
                for n, ek in enumerate([4, 5, 6, 7, 0, 1, 2, 3]):
                    if ek < 4:
                        lhsT = cT_sb[:, ek, i * 128:(i + 1) * 128]
                    else:
                        lhsT = tgtT_sb[:, ek - 4, i * 128:(i + 1) * 128]
                    nc.tensor.matmul(pl, lhsT, wT_sb[:, ek, :],
                                     start=(n == 0), stop=(n == 7))
                ao = outp.tile([128, D], f32, tag="attn_out")
                nc.scalar.copy(out=ao, in_=pl)
                nc.sync.dma_start(out=attnh[b, i * 128:(i + 1) * 128, :], in_=ao)

        # one-batch software pipeline, with pass A(it) and pass B(it-1)
        # interleaved at chunk granularity
        for it in range(B + 1):
            if it < B:
                emit_loads(it)
            if it == 0:
                ident = wp.tile([128, 128], f32)
                make_identity(nc, ident)
                state["ident"] = ident
            if it == 1:
                wT_sb = wp.tile([128, 8, D], f32r)
                nc.sync.dma_start(
                    out=wT_sb,
                    in_=bass.AP(tensor=wT[:, :].tensor, offset=0,
                                ap=[[D, 128], [128 * D, 8], [1, D]]),
                )
                state["wT"] = wT_sb
            if it < B:
                state[it]["negmaxpk"] = st.tile([128, 8], f32, tag="negmaxpk",
                                                name=f"negmaxpk{it}")
                state[it]["rsumpk"] = st.tile([128, 8], f32, tag="rsumpk",
                                              name=f"rsumpk{it}")
            if it > 0:
                state[it - 1]["alignT"] = alt.tile([128, 8, T], f32r, tag="alignT",
                                                   name=f"alignT{it - 1}")
                emit_srcn_load(it - 1)
            for i in range(8):
                if it < B:
                    emit_pass_a_chunk(it, i)
                if it > 0:
                    emit_pass_b_chunk(it - 1, i)
            if it < B:
                emit_adj_chain(it)
            if it > 0:
                b = it - 1
                emit_context(b)
                emit_linear(b)
                del state[b]

    nc.compile()
    return nc


_NC = None


def _get_nc():
    global _NC
    if _NC is None:
        _NC = build()
    return _NC


def make_in_maps(src, tgt, src_lengths, w_out):
    src = np.ascontiguousarray(src, dtype=np.float32)
    tgt = np.ascontiguousarray(tgt, dtype=np.float32)
    srcT = np.ascontiguousarray(src.transpose(0, 2, 1))
    tgtT = np.ascontiguousarray(tgt.transpose(0, 2, 1))
    wT = np.ascontiguousarray(np.asarray(w_out, dtype=np.float32).T)
    lengths = np.asarray(src_lengths)
    mb = np.where(np.arange(S)[None, :] < lengths[:, None], 0.0, NEG).astype(np.float32)
    # mb_pp[b, p, j] = mb[b, j*128 + p]
    mb_pp = np.ascontiguousarray(mb.reshape(BZ, 8, 128).transpose(0, 2, 1))
    in_maps = []
    for c in range(NCORES):
        sl = slice(c * B, (c + 1) * B)
        in_maps.append({
            "srcT": srcT[sl],
            "tgtT": tgtT[sl],
            "srcN": src[sl],
            "wT": wT,
            "mbias": mb[sl],
            "mbias_pp": mb_pp[sl],
        })
    return in_maps


def kernel(src, tgt, src_lengths, w_out, **run_kwargs):
    nc = _get_nc()
    in_maps = make_in_maps(src, tgt, src_lengths, w_out)
    res = run_bass_kernel_spmd(nc, in_maps, list(range(NCORES)), **run_kwargs)
    attn_h = np.concatenate([res.results[c]["attnh"] for c in range(NCORES)], axis=0)
    align = np.concatenate([res.results[c]["align"] for c in range(NCORES)], axis=0)
    if run_kwargs:
        kernel.last_results = res
    return attn_h, align
